# revision 21
# baseline (speedup 1.0000x reference)
"""MHA kernel for Trainium2, 8 NeuronCores — ACT-paced pipeline v3.

Problem: B=4, T=2048, D=1024, H=16, HD=64 fp32 multi-head attention
  qkv = x @ w_qkv ; attention per head ; out = y @ w_o

Sharding: core c handles batch b = c//2 and head-group g = c%2 (8 of the 16
heads). Each core computes its 8 heads' attention output projected through
the matching w_o row-slice, producing a partial [T, D] f16 output; the host
sums the two partials per batch (row-parallel output projection).

Pacing: the scalar (ACT) engine runs ONE combined [128, 2048] exp per slot
(A-head half | B-head half, (2048+352)/1.2 = 2000ns) — 128 slots = 256us.
Per slot g (window w = g//16 = (pair, tb), s-tile i = g%16):

  ACT: exp(g)           reads sc [128,2048] psum (A|B), written slot g-1
  DVE: acc(g) += e(g)   f16 exp-sum, one [128,2048] 2x-mode add
  PE : yu pairs (g-LAG) col-tiled A||B concurrent, into yu [128,1024]
  PE : fills            QKV / O projection chains, budget-paced
  PE : scores(g+1)      4 MMs as 2 row-tiled A||B concurrent pairs
                        (WAR: must follow exp(g) — the only ACT stall)
  PE : fills

At i==15: denominator = ones-matmul of acc in 4 [1,512] chunks through the
aux psum bank -> DVE reciprocal into rec[1,2048] -> gpsimd
partition_broadcast to bc[128,2048] -> one [64,1024] normalize multiply per
head into yt (f16) when that head's yu finishes (LAG slots later).
"""
import sys

if "/opt/trn_rl_repo" not in sys.path:
    sys.path.insert(0, "/opt/trn_rl_repo")

from collections import deque

import numpy as np

import concourse.bass as bass
import concourse.mybir as mybir
import concourse.tile as tile
from concourse import bacc
from concourse.bass_utils import run_bass_kernel_spmd

T = 2048
D = 1024
NH = 8          # heads per core
HD = 64
KC = D // 128   # 8 contraction chunks
TT = T // 128   # 16 s tiles
NP = NH // 2    # 4 head pairs
NW = 2 * NP     # 8 windows: w = 2*p + tb
NG = NW * TT    # 128 global slots
LAG = 4         # yu lags exp by LAG slots
F32 = mybir.dt.float32
F16 = mybir.dt.float16

_CACHE = {}
_DEBUG = False


def build_nc():
    nc = bacc.Bacc(
        "TRN2",
        target_bir_lowering=False,
        debug=False,
        enable_asserts=False,
        num_devices=8,
    )
    x_d = nc.dram_tensor("x", [T, D], F16, kind="ExternalInput")
    wq_d = nc.dram_tensor("wq", [D, 512], F16, kind="ExternalInput")
    wk_d = nc.dram_tensor("wk", [D, 512], F16, kind="ExternalInput")
    wv_d = nc.dram_tensor("wv", [D, 512], F16, kind="ExternalInput")
    wo_d = nc.dram_tensor("wo", [512, D], F16, kind="ExternalInput")
    out_d = nc.dram_tensor("out", [T, D], F16, kind="ExternalOutput")
    if _DEBUG:
        qkt_d = nc.dram_tensor("qkt_dump", [128, 8, T], F16,
                               kind="ExternalOutput")
        v_d = nc.dram_tensor("v_dump", [128, TT, 512], F16,
                             kind="ExternalOutput")
        yt_d = nc.dram_tensor("yt_dump", [128, NP, T], F16,
                              kind="ExternalOutput")

    x_ap = x_d.ap()
    wq_ap = wq_d.ap().rearrange("(kc p) j -> p kc j", p=128)   # [128, 8, 512]
    wk_ap = wk_d.ap().rearrange("(kc p) j -> p kc j", p=128)
    wv_ap = wv_d.ap().rearrange("(kc p) j -> p kc j", p=128)
    wo_ap = wo_d.ap().rearrange("(c p) n -> p c n", p=128)     # [128, 4, 1024]

    def win(g):
        """global slot -> (pair, tb, i)."""
        w, i = g // TT, g % TT
        return w // 2, w % 2, i

    with tile.TileContext(nc) as tc:
        with (
            tc.sbuf_pool(name="sb", bufs=1) as sb,
            tc.psum_pool(name="ps", bufs=1) as ps,
        ):
            # ---- persistent sbuf ----
            xt = sb.tile([128, KC, T], F16)          # x^T  [d, t]
            qkt = sb.tile([128, 8, T], F16)          # jt 0-3 Q^T, 4-7 K^T
            v_sb = sb.tile([128, TT, 512], F16)      # V [s-part, s-chunk, j]
            yt = sb.tile([128, NP, T], F16)          # y^T [dy, pair, t]
            wqk_sb = sb.tile([128, KC, 1024], F16)   # cols 0-511 wq, 512+ wk
            wv_sb = sb.tile([128, KC, 512], F16)
            wo_sb = sb.tile([128, 4, D], F16)
            ones_v = sb.tile([128, 1], F16)
            nc.vector.memset(ones_v, 1.0)
            warm = sb.tile([1, 32], F16)
            nc.vector.memset(warm, 0.0)
            # warm up the ACT exp table before the stream needs it
            nc.scalar.activation(
                warm, warm, mybir.ActivationFunctionType.Exp, scale=0.125
            )

            nc.sync.dma_start(out=wqk_sb[:, :, 512:1024], in_=wk_ap)

            # ---------- fill chains (QKV / O projections) ----------
            fills = deque()
            pending = {}

            def g_qk(jt, tbc):
                """qkt[:, jt, tbc*512:(tbc+1)*512] = (w chunk)^T @ xt."""
                aux = ps.tile([128, 512], F32, name="qkps", tag="aux", bufs=2)
                for kc in range(KC):
                    nc.tensor.matmul(
                        aux,
                        wqk_sb[:, kc, jt * 128:(jt + 1) * 128],
                        xt[:, kc, tbc * 512:(tbc + 1) * 512],
                        start=(kc == 0),
                        stop=(kc == KC - 1),
                        skip_group_check=True,
                    )
                    yield 230
                nc.vector.tensor_copy(
                    out=qkt[:, jt, tbc * 512:(tbc + 1) * 512], in_=aux
                )

            def g_v(i):
                aux = ps.tile([128, 512], F32, name="vps", tag="aux", bufs=2)
                for kc in range(KC):
                    nc.tensor.matmul(
                        aux,
                        xt[:, kc, i * 128:(i + 1) * 128],
                        wv_sb[:, kc, :],
                        start=(kc == 0),
                        stop=(kc == KC - 1),
                        skip_group_check=True,
                    )
                    yield 230
                nc.vector.tensor_copy(out=v_sb[:, i, :], in_=aux)

            def g_o(tt, u):
                aux = ps.tile([128, 512], F32, name="ops", tag="aux", bufs=2)
                for c4 in range(4):
                    nc.tensor.matmul(
                        aux,
                        yt[:, c4, tt * 128:(tt + 1) * 128],
                        wo_sb[:, c4, u * 512:(u + 1) * 512],
                        start=(c4 == 0),
                        stop=(c4 == 3),
                        skip_group_check=True,
                    )
                    yield 230
                o_sb = sb.tile([128, 512], F16, tag="osb", bufs=2)
                with nc.allow_low_precision(reason="f16 partial output"):
                    nc.vector.tensor_copy(out=o_sb, in_=aux)
                nc.sync.dma_start(
                    out=out_d.ap()[
                        tt * 128:(tt + 1) * 128, u * 512:(u + 1) * 512
                    ],
                    in_=o_sb,
                )

            def push_fill(key, gen, front=False):
                pending[key] = gen
                if front:
                    fills.appendleft(key)
                else:
                    fills.append(key)

            def advance_fills(budget):
                while fills and budget > 0:
                    gen = pending.get(fills[0])
                    if gen is None:
                        fills.popleft()
                        continue
                    try:
                        budget -= next(gen)
                    except StopIteration:
                        del pending[fills[0]]
                        fills.popleft()

            def need(key):
                gen = pending.pop(key, None)
                if gen is not None:
                    for _ in gen:
                        pass

            def force_chain(gen):
                for _ in gen:
                    pass

            # ---------- attention state ----------
            # sc: A-half cols 0:1024 (banks 0-1), B-half 1024:2048 (banks 2-3)
            sc = ps.tile([128, 2048], F32, name="sc", tag="sc", bufs=1)
            yu = ps.tile([128, 1024], F32, name="yu", tag="yu", bufs=1)
            exp_t = {}     # g -> sbuf exp tile [128, 2048] (A|B)
            acc_t = [None]  # running f16 exp-sum tile [128, 2048]
            bc_t = {}      # w -> broadcast 1/denominator tile [128, 2048]

            def emit_sc(g):
                """scores for slot g: 4 MMs as 2 row-tiled A||B pairs."""
                if g < 0 or g >= NG:
                    return
                p, tb, i = win(g)
                need(("qk", 4 + p, i // 4))
                need(("qk", p, 2 * tb))
                need(("qk", p, 2 * tb + 1))
                for u in range(2):
                    for hb in range(2):   # A then B adjacent -> concurrent
                        pb = 64 * hb
                        nc.tensor.matmul(
                            sc[:, 1024 * hb + u * 512:
                               1024 * hb + (u + 1) * 512],
                            qkt[pb:pb + 64, 4 + p, i * 128:(i + 1) * 128],
                            qkt[pb:pb + 64, p,
                                tb * 1024 + u * 512:tb * 1024 + (u + 1) * 512],
                            start=True,
                            stop=True,
                        )

            def emit_exp(g):
                if g < 0 or g >= NG:
                    return
                e = sb.tile([128, 2048], F16, tag="exp", bufs=6)
                nc.scalar.activation(
                    e, sc, mybir.ActivationFunctionType.Exp, scale=0.125
                )
                exp_t[g] = e

            def emit_acc(g):
                if g < 0 or g >= NG:
                    return
                i = g % TT
                a = sb.tile([128, 2048], F16, tag="acc", bufs=2)
                if i == 0:
                    nc.vector.tensor_copy(out=a, in_=exp_t[g])
                else:
                    with nc.allow_low_precision(reason="f16 exp-sum"):
                        nc.vector.tensor_add(out=a, in0=acc_t[0], in1=exp_t[g])
                acc_t[0] = a
                if i == TT - 1:
                    # denominator: 4x [1,512] ones-matmul chunks via aux,
                    # reciprocal into rec, broadcast to bc.
                    w = g // TT
                    rec = sb.tile([1, 2048], F32, tag="rec", bufs=1)
                    bc = sb.tile([128, 2048], F32, tag="bc", bufs=1)
                    for c in range(4):
                        dn = ps.tile([128, 512], F32, name="dn",
                                     tag="aux", bufs=2)
                        nc.tensor.matmul(
                            dn[0:1, :],
                            ones_v,
                            a[:, c * 512:(c + 1) * 512],
                            start=True,
                            stop=True,
                            tile_position=(0, 0),
                        )
                        nc.vector.reciprocal_approx_fast(
                            out=rec[0:1, c * 512:(c + 1) * 512],
                            in_=dn[0:1, :],
                        )
                        nc.gpsimd.partition_broadcast(
                            bc[:, c * 512:(c + 1) * 512],
                            rec[0:1, c * 512:(c + 1) * 512],
                            channels=128,
                        )
                    bc_t[w] = bc

            def emit_yu(g):
                """col-tiled concurrent pairs: yu_A(g) || yu_B(g)."""
                if g < 0 or g >= NG:
                    return
                p, tb, i = win(g)
                w = g // TT
                need(("v", i))
                e = exp_t.pop(g)
                for u in range(2):
                    for hb in range(2):   # A then B adjacent -> concurrent
                        pb = 64 * hb
                        nc.tensor.matmul(
                            yu[pb:pb + 64, u * 512:(u + 1) * 512],
                            v_sb[:, i, 128 * p + pb:128 * p + pb + 64],
                            e[:, 1024 * hb + u * 512:1024 * hb + (u + 1) * 512],
                            start=(i == 0),
                            stop=(i == TT - 1),
                            skip_group_check=True,
                        )
                if i == TT - 1:
                    # normalize both heads into yt
                    bc = bc_t.pop(w)
                    with nc.allow_low_precision(reason="f16 y"):
                        for hb in range(2):
                            pb = 64 * hb
                            nc.vector.tensor_mul(
                                out=yt[pb:pb + 64, p,
                                       tb * 1024:(tb + 1) * 1024],
                                in0=yu[pb:pb + 64, :],
                                in1=bc[pb:pb + 64,
                                       1024 * hb:1024 * (hb + 1)],
                            )

            # ---------- startup ----------
            # t 0:512 transposes first: they gate K(4,0) and Q(0,0)
            for kc in range(KC):
                nc.sync.dma_start_transpose(
                    out=xt[:, kc, 0:512],
                    in_=x_ap[0:512, kc * 128:(kc + 1) * 128],
                )
            nc.sync.dma_start(out=wqk_sb[:, :, 0:512], in_=wq_ap)
            for kc in range(KC):
                nc.sync.dma_start_transpose(
                    out=xt[:, kc, 512:1024],
                    in_=x_ap[512:1024, kc * 128:(kc + 1) * 128],
                )
            nc.sync.dma_start(out=wv_sb, in_=wv_ap)
            force_chain(g_qk(4, 0))   # K^T pair 0, s 0:512
            force_chain(g_qk(0, 0))   # Q^T pair 0, t 0:512
            force_chain(g_qk(0, 1))   # Q^T pair 0, t 512:1024
            v_gens = {i: g_v(i) for i in range(TT)}
            for i_ in range(4):
                force_chain(v_gens.pop(i_))

            def xpose2(kc):
                nc.sync.dma_start_transpose(
                    out=xt[:, kc, 1024:2048],
                    in_=x_ap[1024:2048, kc * 128:(kc + 1) * 128],
                )

            startup_forced = {
                0: [lambda: [xpose2(kc) for kc in range(4)]],
                1: [lambda: [xpose2(kc) for kc in range(4, KC)],
                    lambda: force_chain(g_qk(4, 1))],
                2: [lambda: nc.sync.dma_start(out=wo_sb, in_=wo_ap)],
                4: [lambda: force_chain(g_qk(4, 2))],
                6: [lambda: force_chain(g_qk(4, 3))],
            }

            def fill_pushes(g):
                """push new fill chains at window starts."""
                w, i = g // TT, g % TT
                if i == 0:
                    p, tb = w // 2, w % 2
                    if p < 3:
                        jt = (p + 1) if tb == 0 else (4 + p + 1)
                        for tbc in reversed(range(4)):
                            push_fill(("qk", jt, tbc), g_qk(jt, tbc),
                                      front=True)
                    if w == 0:
                        push_fill(("qk", 0, 3), g_qk(0, 3), front=True)
                        push_fill(("qk", 0, 2), g_qk(0, 2), front=True)
                # o(tb0) chains read yt pair-3 tb0, normalized at slot
                # 16*6+15+LAG; push strictly after.
                if g == 16 * 6 + 15 + LAG + 1:
                    for tt in range(8):
                        for u in range(2):
                            push_fill(("o", tt, u), g_o(tt, u))

            # v chains keyed for need(); stream as priority fills
            for i_ in sorted(v_gens):
                pending[("v", i_)] = v_gens[i_]
                fills.append(("v", i_))

            # sc(0) must exist before exp(0)
            emit_sc(0)

            # ---------- main loop ----------
            for g in range(NG + LAG + 1):
                first_win = g < TT
                if g < NG:
                    fill_pushes(g)
                emit_exp(g)
                emit_acc(g)
                emit_yu(g - LAG)
                if first_win:
                    for fn in startup_forced.get(g, ()):
                        fn()
                advance_fills(300 if first_win else 500)
                emit_sc(g + 1)
                advance_fills(300 if first_win else 500)

            # ---------- tail: output projection for tb=1 ----------
            while fills:
                advance_fills(10000)
            for tt in range(8, 16):
                for u in range(2):
                    force_chain(g_o(tt, u))
            if _DEBUG:
                nc.sync.dma_start(out=qkt_d.ap(), in_=qkt)
                nc.sync.dma_start(out=v_d.ap(), in_=v_sb)
                nc.sync.dma_start(out=yt_d.ap(), in_=yt)

    nc.compile()
    return nc


def make_in_maps(x, w_qkv, w_o):
    in_maps = []
    for c in range(8):
        b, gg = c // 2, c % 2
        in_maps.append({
            "x": np.ascontiguousarray(x[b], dtype=np.float16),
            "wq": np.ascontiguousarray(
                w_qkv[:, 512 * gg:512 * (gg + 1)], dtype=np.float16),
            "wk": np.ascontiguousarray(
                w_qkv[:, 1024 + 512 * gg:1024 + 512 * (gg + 1)],
                dtype=np.float16),
            "wv": np.ascontiguousarray(
                w_qkv[:, 2048 + 512 * gg:2048 + 512 * (gg + 1)],
                dtype=np.float16),
            "wo": np.ascontiguousarray(
                w_o[512 * gg:512 * (gg + 1), :], dtype=np.float16),
        })
    return in_maps


def kernel(x, w_qkv, w_o, _trace=False, _trace_kwargs=None):
    x = np.asarray(x)
    w_qkv = np.asarray(w_qkv)
    w_o = np.asarray(w_o)
    if "nc" not in _CACHE:
        _CACHE["nc"] = build_nc()
    nc = _CACHE["nc"]
    in_maps = make_in_maps(x, w_qkv, w_o)
    res = run_bass_kernel_spmd(
        nc, in_maps, core_ids=list(range(8)),
        trace=_trace, **(_trace_kwargs or {}),
    )
    out = np.empty((4, T, D), np.float32)
    for b in range(4):
        out[b] = (res.results[2 * b]["out"].astype(np.float32)
                  + res.results[2 * b + 1]["out"].astype(np.float32))
    if _trace:
        _CACHE["last_res"] = res
    return out


# revision 30
# speedup vs baseline: 1.0139x; 1.0139x over previous
"""MHA kernel for Trainium2, 8 NeuronCores — ACT-paced pipeline v3.

Problem: B=4, T=2048, D=1024, H=16, HD=64 fp32 multi-head attention
  qkv = x @ w_qkv ; attention per head ; out = y @ w_o

Sharding: core c handles batch b = c//2 and head-group g = c%2 (8 of the 16
heads). Each core computes its 8 heads' attention output projected through
the matching w_o row-slice, producing a partial [T, D] f16 output; the host
sums the two partials per batch (row-parallel output projection).

Pacing: the scalar (ACT) engine runs ONE combined [128, 2048] exp per slot
(A-head half | B-head half, (2048+352)/1.2 = 2000ns) — 128 slots = 256us.
Per slot g (window w = g//16 = (pair, tb), s-tile i = g%16):

  ACT: exp(g)           reads sc [128,2048] psum (A|B), written slot g-1
  DVE: acc(g) += e(g)   f16 exp-sum, one [128,2048] 2x-mode add
  PE : yu pairs (g-LAG) col-tiled A||B concurrent, into yu [128,1024]
  PE : fills            QKV / O projection chains, budget-paced
  PE : scores(g+1)      4 MMs as 2 row-tiled A||B concurrent pairs
                        (WAR: must follow exp(g) — the only ACT stall)
  PE : fills

At i==15: denominator = ones-matmul of acc in 4 [1,512] chunks through the
aux psum bank -> DVE reciprocal into rec[1,2048] -> gpsimd
partition_broadcast to bc[128,2048] -> one [64,1024] normalize multiply per
head into yt (f16) when that head's yu finishes (LAG slots later).
"""
import sys

if "/opt/trn_rl_repo" not in sys.path:
    sys.path.insert(0, "/opt/trn_rl_repo")

from collections import deque

import numpy as np

import concourse.bass as bass
import concourse.mybir as mybir
import concourse.tile as tile
from concourse import bacc
from concourse.bass_utils import run_bass_kernel_spmd

T = 2048
D = 1024
NH = 8          # heads per core
HD = 64
KC = D // 128   # 8 contraction chunks
TT = T // 128   # 16 s tiles
NP = NH // 2    # 4 head pairs
NW = 2 * NP     # 8 windows: w = 2*p + tb
NG = NW * TT    # 128 global slots
LAG = 4         # yu lags exp by LAG slots
F32 = mybir.dt.float32
F16 = mybir.dt.float16

_CACHE = {}
_DEBUG = False


def build_nc():
    nc = bacc.Bacc(
        "TRN2",
        target_bir_lowering=False,
        debug=False,
        enable_asserts=False,
        num_devices=8,
    )
    x_d = nc.dram_tensor("x", [T, D], F16, kind="ExternalInput")
    wq_d = nc.dram_tensor("wq", [D, 512], F16, kind="ExternalInput")
    wk_d = nc.dram_tensor("wk", [D, 512], F16, kind="ExternalInput")
    wv_d = nc.dram_tensor("wv", [D, 512], F16, kind="ExternalInput")
    wo_d = nc.dram_tensor("wo", [512, D], F16, kind="ExternalInput")
    out_d = nc.dram_tensor("out", [T, D], F16, kind="ExternalOutput")
    if _DEBUG:
        qkt_d = nc.dram_tensor("qkt_dump", [128, 8, T], F16,
                               kind="ExternalOutput")
        v_d = nc.dram_tensor("v_dump", [128, TT, 512], F16,
                             kind="ExternalOutput")
        yt_d = nc.dram_tensor("yt_dump", [128, NP, T], F16,
                              kind="ExternalOutput")

    x_ap = x_d.ap()
    wq_ap = wq_d.ap().rearrange("(kc p) j -> p kc j", p=128)   # [128, 8, 512]
    wk_ap = wk_d.ap().rearrange("(kc p) j -> p kc j", p=128)
    wv_ap = wv_d.ap().rearrange("(kc p) j -> p kc j", p=128)
    wo_ap = wo_d.ap().rearrange("(c p) n -> p c n", p=128)     # [128, 4, 1024]

    def win(g):
        """global slot -> (pair, tb, i)."""
        w, i = g // TT, g % TT
        return w // 2, w % 2, i

    with tile.TileContext(nc) as tc:
        with (
            tc.sbuf_pool(name="sb", bufs=1) as sb,
            tc.psum_pool(name="ps", bufs=1) as ps,
        ):
            # ---- persistent sbuf ----
            xt = sb.tile([128, KC, T], F16)          # x^T  [d, t]
            qkt = sb.tile([128, 8, T], F16)          # jt 0-3 Q^T, 4-7 K^T
            v_sb = sb.tile([128, TT, 512], F16)      # V [s-part, s-chunk, j]
            yt = sb.tile([128, NP, T], F16)          # y^T [dy, pair, t]
            wqk_sb = sb.tile([128, KC, 1024], F16)   # cols 0-511 wq, 512+ wk
            wv_sb = sb.tile([128, KC, 512], F16)
            wo_sb = sb.tile([128, 4, D], F16)
            ones_v = sb.tile([128, 1], F16)
            nc.vector.memset(ones_v, 1.0)
            warm = sb.tile([1, 32], F16)
            nc.vector.memset(warm, 0.0)
            # warm up the ACT exp table before the stream needs it
            nc.scalar.activation(
                warm, warm, mybir.ActivationFunctionType.Exp, scale=0.125
            )

            nc.scalar.dma_start(out=wqk_sb[:, :, 512:1024], in_=wk_ap)

            # ---------- fill chains (QKV / O projections) ----------
            # deadline-ordered: advance_fills always works on the chain
            # whose consumer slot comes first.
            import heapq
            fills = []     # heap of (deadline_slot, seq, key)
            fseq = [0]
            pending = {}

            def g_qk(jt, tbc):
                """qkt[:, jt, tbc*512:(tbc+1)*512] = (w chunk)^T @ xt."""
                aux = ps.tile([128, 512], F32, name="qkps", tag="aux", bufs=2)
                for kc in range(KC):
                    nc.tensor.matmul(
                        aux,
                        wqk_sb[:, kc, jt * 128:(jt + 1) * 128],
                        xt[:, kc, tbc * 512:(tbc + 1) * 512],
                        start=(kc == 0),
                        stop=(kc == KC - 1),
                        skip_group_check=True,
                    )
                    yield 230
                nc.vector.tensor_copy(
                    out=qkt[:, jt, tbc * 512:(tbc + 1) * 512], in_=aux
                )

            def g_v(i):
                aux = ps.tile([128, 512], F32, name="vps", tag="aux", bufs=2)
                for kc in range(KC):
                    nc.tensor.matmul(
                        aux,
                        xt[:, kc, i * 128:(i + 1) * 128],
                        wv_sb[:, kc, :],
                        start=(kc == 0),
                        stop=(kc == KC - 1),
                        skip_group_check=True,
                    )
                    yield 230
                nc.vector.tensor_copy(out=v_sb[:, i, :], in_=aux)

            def g_o(tt, u):
                aux = ps.tile([128, 512], F32, name="ops", tag="aux", bufs=2)
                for c4 in range(4):
                    nc.tensor.matmul(
                        aux,
                        yt[:, c4, tt * 128:(tt + 1) * 128],
                        wo_sb[:, c4, u * 512:(u + 1) * 512],
                        start=(c4 == 0),
                        stop=(c4 == 3),
                        skip_group_check=True,
                    )
                    yield 230
                o_sb = sb.tile([128, 512], F16, tag="osb", bufs=2)
                with nc.allow_low_precision(reason="f16 partial output"):
                    nc.vector.tensor_copy(out=o_sb, in_=aux)
                nc.sync.dma_start(
                    out=out_d.ap()[
                        tt * 128:(tt + 1) * 128, u * 512:(u + 1) * 512
                    ],
                    in_=o_sb,
                )

            def push_fill(key, gen, deadline=10000):
                pending[key] = gen
                heapq.heappush(fills, (deadline, fseq[0], key))
                fseq[0] += 1

            def advance_fills(budget):
                while fills and budget > 0:
                    key = fills[0][2]
                    gen = pending.get(key)
                    if gen is None:
                        heapq.heappop(fills)
                        continue
                    try:
                        budget -= next(gen)
                    except StopIteration:
                        del pending[key]
                        heapq.heappop(fills)

            def need(key):
                gen = pending.pop(key, None)
                if gen is not None:
                    for _ in gen:
                        pass

            def force_chain(gen):
                for _ in gen:
                    pass

            # ---------- attention state ----------
            # sc: A-half cols 0:1024 (banks 0-1), B-half 1024:2048 (banks 2-3)
            sc = ps.tile([128, 2048], F32, name="sc", tag="sc", bufs=1)
            yu = ps.tile([128, 1024], F32, name="yu", tag="yu", bufs=1)
            exp_t = {}     # g -> sbuf exp tile [128, 2048] (A|B)
            acc_t = [None]  # running f16 exp-sum tile [128, 2048]
            bc_t = {}      # w -> broadcast 1/denominator tile [128, 2048]

            def emit_sc(g):
                """scores for slot g: 4 MMs as 2 row-tiled A||B pairs."""
                if g < 0 or g >= NG:
                    return
                p, tb, i = win(g)
                need(("qk", 4 + p, i // 4))
                need(("qk", p, 2 * tb))
                need(("qk", p, 2 * tb + 1))
                for u in range(2):
                    for hb in range(2):   # A then B adjacent -> concurrent
                        pb = 64 * hb
                        nc.tensor.matmul(
                            sc[:, 1024 * hb + u * 512:
                               1024 * hb + (u + 1) * 512],
                            qkt[pb:pb + 64, 4 + p, i * 128:(i + 1) * 128],
                            qkt[pb:pb + 64, p,
                                tb * 1024 + u * 512:tb * 1024 + (u + 1) * 512],
                            start=True,
                            stop=True,
                        )

            def emit_exp(g):
                if g < 0 or g >= NG:
                    return
                e = sb.tile([128, 2048], F16, tag="exp", bufs=6)
                nc.scalar.activation(
                    e, sc, mybir.ActivationFunctionType.Exp, scale=0.125
                )
                exp_t[g] = e

            def emit_acc(g):
                if g < 0 or g >= NG:
                    return
                i = g % TT
                a = sb.tile([128, 2048], F16, tag="acc", bufs=2)
                if i == 0:
                    nc.vector.tensor_copy(out=a, in_=exp_t[g])
                else:
                    with nc.allow_low_precision(reason="f16 exp-sum"):
                        nc.vector.tensor_add(out=a, in0=acc_t[0], in1=exp_t[g])
                acc_t[0] = a
                if i == TT - 1:
                    # denominator: 4x [1,512] ones-matmul chunks via aux,
                    # reciprocal into rec, broadcast to bc.
                    w = g // TT
                    rec = sb.tile([1, 2048], F32, tag="rec", bufs=1)
                    bc = sb.tile([128, 2048], F32, tag="bc", bufs=1)
                    for c in range(4):
                        dn = ps.tile([128, 512], F32, name="dn",
                                     tag="aux", bufs=2)
                        nc.tensor.matmul(
                            dn[0:1, :],
                            ones_v,
                            a[:, c * 512:(c + 1) * 512],
                            start=True,
                            stop=True,
                            tile_position=(0, 0),
                        )
                        nc.vector.reciprocal_approx_fast(
                            out=rec[0:1, c * 512:(c + 1) * 512],
                            in_=dn[0:1, :],
                        )
                        nc.gpsimd.partition_broadcast(
                            bc[:, c * 512:(c + 1) * 512],
                            rec[0:1, c * 512:(c + 1) * 512],
                            channels=128,
                        )
                    bc_t[w] = bc

            def emit_yu(g):
                """col-tiled concurrent pairs: yu_A(g) || yu_B(g)."""
                if g < 0 or g >= NG:
                    return
                p, tb, i = win(g)
                w = g // TT
                need(("v", i))
                e = exp_t.pop(g)
                for u in range(2):
                    for hb in range(2):   # A then B adjacent -> concurrent
                        pb = 64 * hb
                        nc.tensor.matmul(
                            yu[pb:pb + 64, u * 512:(u + 1) * 512],
                            v_sb[:, i, 128 * p + pb:128 * p + pb + 64],
                            e[:, 1024 * hb + u * 512:1024 * hb + (u + 1) * 512],
                            start=(i == 0),
                            stop=(i == TT - 1),
                            skip_group_check=True,
                        )
                if i == TT - 1:
                    # normalize both heads into yt
                    bc = bc_t.pop(w)
                    with nc.allow_low_precision(reason="f16 y"):
                        for hb in range(2):
                            pb = 64 * hb
                            nc.vector.tensor_mul(
                                out=yt[pb:pb + 64, p,
                                       tb * 1024:(tb + 1) * 1024],
                                in0=yu[pb:pb + 64, :],
                                in1=bc[pb:pb + 64,
                                       1024 * hb:1024 * (hb + 1)],
                            )

            # ---------- startup ----------
            # t 0:512 transposes first: they gate K(4,0) and Q(0,0)
            for kc in range(KC):
                nc.sync.dma_start_transpose(
                    out=xt[:, kc, 0:512],
                    in_=x_ap[0:512, kc * 128:(kc + 1) * 128],
                )
            nc.scalar.dma_start(out=wqk_sb[:, :, 0:512], in_=wq_ap)
            for kc in range(KC):
                nc.sync.dma_start_transpose(
                    out=xt[:, kc, 512:1024],
                    in_=x_ap[512:1024, kc * 128:(kc + 1) * 128],
                )
            nc.gpsimd.dma_start(out=wv_sb, in_=wv_ap)
            force_chain(g_qk(4, 0))   # K^T pair 0, s 0:512
            force_chain(g_qk(0, 0))   # Q^T pair 0, t 0:512
            force_chain(g_qk(0, 1))   # Q^T pair 0, t 512:1024
            v_gens = {i: g_v(i) for i in range(TT)}
            for i_ in range(4):
                force_chain(v_gens.pop(i_))

            def xpose2(kc):
                nc.sync.dma_start_transpose(
                    out=xt[:, kc, 1024:2048],
                    in_=x_ap[1024:2048, kc * 128:(kc + 1) * 128],
                )

            startup_forced = {
                0: [lambda: [xpose2(kc) for kc in range(4)]],
                1: [lambda: [xpose2(kc) for kc in range(4, KC)],
                    lambda: force_chain(g_qk(4, 1))],
                2: [lambda: nc.gpsimd.dma_start(out=wo_sb, in_=wo_ap)],
                4: [lambda: force_chain(g_qk(4, 2))],
                6: [lambda: force_chain(g_qk(4, 3))],
            }

            def fill_pushes(g):
                """push new fill chains at window starts."""
                w, i = g // TT, g % TT
                if i == 0:
                    p, tb = w // 2, w % 2
                    if p < 3:
                        # chains for windows w+2 (same-kind Q) / w+1 (K):
                        # Q(p+1) feeds window 2(p+1); K(p+1) feeds 2(p+1) too
                        jt = (p + 1) if tb == 0 else (4 + p + 1)
                        base = 16 * (2 * (p + 1))      # first consuming slot
                        for tbc in range(4):
                            if jt < 4:   # Q chain: tbc pairs gate window tb
                                dl = base + (0 if tbc < 2 else 16) - 2
                            else:        # K chain: tbc c gates s-tile 4c
                                dl = base + 4 * tbc - 2
                            push_fill(("qk", jt, tbc), g_qk(jt, tbc),
                                      deadline=dl)
                    if w == 0:
                        push_fill(("qk", 0, 2), g_qk(0, 2), deadline=13)
                        push_fill(("qk", 0, 3), g_qk(0, 3), deadline=14)
                # o(tb0) chains read yt pair-3 tb0, normalized at slot
                # 16*6+15+LAG; push strictly after.
                if g == 16 * 6 + 15 + LAG + 1:
                    for tt in range(8):
                        for u in range(2):
                            push_fill(("o", tt, u), g_o(tt, u),
                                      deadline=500)

            # v chains keyed for need(); deadline = consuming yu slot
            for i_ in sorted(v_gens):
                push_fill(("v", i_), v_gens[i_], deadline=i_ + LAG - 1)

            # sc(0) must exist before exp(0)
            emit_sc(0)

            # ---------- main loop ----------
            for g in range(NG + LAG + 1):
                first_win = g < TT
                if g < NG:
                    fill_pushes(g)
                emit_exp(g)
                emit_acc(g)
                emit_yu(g - LAG)
                if first_win:
                    for fn in startup_forced.get(g, ()):
                        fn()
                advance_fills(250 if first_win else 420)
                emit_sc(g + 1)
                advance_fills(250 if first_win else 430)

            # ---------- tail: output projection for tb=1 ----------
            while fills:
                advance_fills(10000)
            for tt in range(8, 16):
                for u in range(2):
                    force_chain(g_o(tt, u))
            if _DEBUG:
                nc.sync.dma_start(out=qkt_d.ap(), in_=qkt)
                nc.sync.dma_start(out=v_d.ap(), in_=v_sb)
                nc.sync.dma_start(out=yt_d.ap(), in_=yt)

    nc.compile()
    return nc


def make_in_maps(x, w_qkv, w_o):
    in_maps = []
    for c in range(8):
        b, gg = c // 2, c % 2
        in_maps.append({
            "x": np.ascontiguousarray(x[b], dtype=np.float16),
            "wq": np.ascontiguousarray(
                w_qkv[:, 512 * gg:512 * (gg + 1)], dtype=np.float16),
            "wk": np.ascontiguousarray(
                w_qkv[:, 1024 + 512 * gg:1024 + 512 * (gg + 1)],
                dtype=np.float16),
            "wv": np.ascontiguousarray(
                w_qkv[:, 2048 + 512 * gg:2048 + 512 * (gg + 1)],
                dtype=np.float16),
            "wo": np.ascontiguousarray(
                w_o[512 * gg:512 * (gg + 1), :], dtype=np.float16),
        })
    return in_maps


def kernel(x, w_qkv, w_o, _trace=False, _trace_kwargs=None):
    x = np.asarray(x)
    w_qkv = np.asarray(w_qkv)
    w_o = np.asarray(w_o)
    if "nc" not in _CACHE:
        _CACHE["nc"] = build_nc()
    nc = _CACHE["nc"]
    in_maps = make_in_maps(x, w_qkv, w_o)
    res = run_bass_kernel_spmd(
        nc, in_maps, core_ids=list(range(8)),
        trace=_trace, **(_trace_kwargs or {}),
    )
    out = np.empty((4, T, D), np.float32)
    for b in range(4):
        out[b] = (res.results[2 * b]["out"].astype(np.float32)
                  + res.results[2 * b + 1]["out"].astype(np.float32))
    if _trace:
        _CACHE["last_res"] = res
    return out
